# revision 3
# baseline (speedup 1.0000x reference)
"""Trainium2 Bass kernel for nn_CorrelationImage.

reference:
    corr_b = sum(map1[b] * map2[b])            # dot over C*H*W per sample
    corr   = corr / ||corr||_2                 # L2 norm over the batch
    out    = map1 + map2 * (1 - corr)[:, None, None, None]

Sharding: data-parallel over batch B=64 across 8 cores (8 samples/core).
Per core:
  1. stream the 8 (map1, map2) sample pairs into SBUF (kept resident);
     a dummy 4B AllReduce posted at entry absorbs the NRT collective
     bootstrap (~40-55us) while the loads stream and synchronizes the
     cores so the real AllReduce runs hot (~9us),
  2. per-sample dot: DVE multiply + free-dim reduce (samples 0-5 in
     pairs, 6 and 7 singly for a short post-load tail), then a
     ones-matmul partition reduce -> c_i replicated on 128 partitions,
  3. the local sum of squares is kept replicated per partition and
     AllReduce-added as a [128] vector, so the global ss arrives
     already partition-shaped: post-AR work is just DMA-in, Sqrt,
     reciprocal, and one fused t_i = c_i*inv - 1 (= -(1 - c~_i)),
  4. out_i = map1_i - map2_i * t_i in place (ScalarE scale + DVE sub),
     sample 0 in quarter-tiles and sample 1 in halves so the first
     store fires ~1.5us after the norm is known; stores at ~357 GB/s.
"""

import sys

import numpy as np

if "/opt/trn_rl_repo" not in sys.path:
    sys.path.insert(0, "/opt/trn_rl_repo")

B, C, H, W = 64, 64, 64, 64
N_CORES = 8
SPC = B // N_CORES  # samples per core
PART = 128
ELEMS = C * H * W  # 262144 per sample
FD = ELEMS // PART  # 2048 free-dim per sample tile

_cache = {}


def _build_nc(spc=SPC, fd=FD, n_cores=N_CORES, use_cc=True, cc_shared=True):
    from contextlib import ExitStack

    from concourse import bacc, tile, mybir

    f32 = mybir.dt.float32
    Alu = mybir.AluOpType
    Act = mybir.ActivationFunctionType

    nc = bacc.Bacc(
        "TRN2", target_bir_lowering=False, debug=False, num_devices=n_cores
    )
    m1d = nc.dram_tensor("map1", [spc, PART, fd], f32, kind="ExternalInput").ap()
    m2d = nc.dram_tensor("map2", [spc, PART, fd], f32, kind="ExternalInput").ap()
    outd = nc.dram_tensor("out", [spc, PART, fd], f32, kind="ExternalOutput").ap()

    with tile.TileContext(nc) as tc, ExitStack() as ctx:
        big = ctx.enter_context(tc.tile_pool(name="big", bufs=1))
        scratch = ctx.enter_context(tc.tile_pool(name="scratch", bufs=2))
        small = ctx.enter_context(tc.tile_pool(name="small", bufs=1))
        psum = ctx.enter_context(tc.tile_pool(name="psum", bufs=1, space="PSUM"))
        dram = ctx.enter_context(tc.tile_pool(name="dram", bufs=1, space="DRAM"))

        m1s = big.tile([PART, spc * fd], f32)
        m2s = big.tile([PART, spc * fd], f32)

        # dummy 4B AllReduce issued at entry: absorbs the NRT collective
        # bootstrap (barrier + ring setup) while the loads stream in, so
        # the real AllReduce below runs hot.  Dedicated pool + zeroed input
        # so it can never alias or poison the real collective's buffers.
        wdram = ctx.enter_context(tc.tile_pool(name="wdram", bufs=1, space="DRAM"))
        wcc_in = wdram.tile([1], f32)
        wcc_out = wdram.tile([1], f32)
        z1 = small.tile([1, 1], f32)
        nc.vector.memset(z1, 0.0)
        nc.sync.dma_start(out=wcc_in[:], in_=z1[:])
        nc.gpsimd.collective_compute(
            "AllReduce",
            Alu.add,
            replica_groups=[list(range(n_cores))],
            ins=[wcc_in.opt()],
            outs=[wcc_out.opt()],
        )

        for i in range(spc):
            nc.sync.dma_start(out=m1s[:, i * fd : (i + 1) * fd], in_=m1d[i])
            nc.sync.dma_start(out=m2s[:, i * fd : (i + 1) * fd], in_=m2d[i])

        ones_t = small.tile([PART, PART], f32)
        nc.vector.memset(ones_t, 1.0)
        partials = small.tile([PART, spc], f32)
        # preload the sqrt table set off the critical path (Copy rides along)
        warm = small.tile([1, 1], f32)
        nc.vector.memset(warm, 1.0)
        nc.scalar.activation(out=warm, in_=warm, func=Act.Sqrt)

        # per-sample dot: process samples in PAIRS entirely on DVE — one
        # [128, 2*fd] multiply then one 3D tensor_reduce into two partials
        # columns; chained DVE ops issue back-to-back, so each pair costs
        # ~2x(2*fd) cycles with almost no per-op gap and tracks the loads
        groups = [(0, 2), (2, 2), (4, 2), (6, 1), (7, 1)]
        for g0, glen in groups:
            prod = scratch.tile([PART, glen, fd], f32, name="prod")
            sl = slice(g0 * fd, (g0 + glen) * fd)
            nc.vector.tensor_mul(
                out=prod.rearrange("p a f -> p (a f)"),
                in0=m1s[:, sl],
                in1=m2s[:, sl],
            )
            nc.vector.tensor_reduce(
                out=partials[:, g0 : g0 + glen],
                in_=prod,
                axis=mybir.AxisListType.X,
                op=Alu.add,
            )

        # partition reduce; c_i replicated across all 128 partitions
        c8 = psum.tile([PART, spc], f32)
        nc.tensor.matmul(c8, ones_t, partials, start=True, stop=True)
        # local sum of squares of the 8 dots, kept replicated on all 128
        # partitions so the AllReduce payload is already partition-shaped:
        # AllReduce-add of [128] (one copy per partition) -> global ss
        # replicated, with no post-AR reduce or matmul on the critical path
        c8s = small.tile([PART, spc], f32)
        nc.vector.tensor_copy(out=c8s, in_=c8)
        csq8 = small.tile([PART, spc], f32)
        nc.vector.tensor_mul(out=csq8, in0=c8s, in1=c8s)
        ssl = small.tile([PART, 1], f32)
        nc.vector.tensor_reduce(
            out=ssl, in_=csq8, axis=mybir.AxisListType.X, op=Alu.add
        )
        cc_in = dram.tile([PART], f32)
        nc.sync.dma_start(out=cc_in[:], in_=ssl[:])
        gssp = small.tile([PART, 1], f32)
        if use_cc:
            cc_out = dram.tile(
                [PART],
                f32,
                addr_space="Shared" if (cc_shared and n_cores > 4) else "Local",
            )
            nc.gpsimd.collective_compute(
                "AllReduce",
                Alu.add,
                replica_groups=[list(range(n_cores))],
                ins=[cc_in.opt()],
                outs=[cc_out.opt()],
            )
            nc.sync.dma_start(out=gssp[:], in_=cc_out[:])
        else:
            # debug only: pretend every core holds the same 8 samples
            nc.vector.tensor_scalar_mul(out=gssp, in0=ssl, scalar1=float(n_cores))

        # t_i = c_i*rsqrt(ss) - 1 = -(1 - c~_i); epilogue computes
        # out = map1 - map2*t so no separate negate is needed
        normb = small.tile([PART, 1], f32)
        nc.scalar.activation(out=normb, in_=gssp, func=Act.Sqrt)
        inv = small.tile([PART, 1], f32)
        nc.vector.reciprocal(out=inv, in_=normb)
        t8 = small.tile([PART, spc], f32)
        nc.vector.tensor_scalar(
            out=t8,
            in0=c8,
            scalar1=inv,
            scalar2=1.0,
            op0=Alu.mult,
            op1=Alu.subtract,
        )

        # out_i = map2_i * s_i + map1_i, fully in place in the map2 buffer;
        # ScalarE does the per-sample scale (clean 2 us pace), the adds run
        # on DVE over PAIRS of adjacent samples (halves the per-op drain
        # tax), stores stream out per sample
        def chunk(i, c0, clen):
            sl = slice(i * fd + c0, i * fd + c0 + clen)
            nc.scalar.activation(
                out=m2s[:, sl],
                in_=m2s[:, sl],
                func=Act.Copy,
                scale=t8[:, i : i + 1],
            )
            nc.vector.tensor_sub(out=m2s[:, sl], in0=m1s[:, sl], in1=m2s[:, sl])
            nc.sync.dma_start(out=outd[i][:, c0 : c0 + clen], in_=m2s[:, sl])

        q = fd // 4
        for j in range(4):
            chunk(0, j * q, q)
        h = fd // 2
        for j in range(2):
            chunk(1, j * h, h)
        for i in range(2, spc):
            sl = slice(i * fd, (i + 1) * fd)
            nc.scalar.activation(
                out=m2s[:, sl],
                in_=m2s[:, sl],
                func=Act.Copy,
                scale=t8[:, i : i + 1],
            )
            if i % 2 == 1:
                psl = slice((i - 1) * fd, (i + 1) * fd)
                nc.vector.tensor_sub(
                    out=m2s[:, psl], in0=m1s[:, psl], in1=m2s[:, psl]
                )
                nc.sync.dma_start(out=outd[i - 1], in_=m2s[:, (i - 1) * fd : i * fd])
                nc.sync.dma_start(out=outd[i], in_=m2s[:, sl])

    nc.compile()
    return nc


def _get_nc():
    if "nc" not in _cache:
        _cache["nc"] = _build_nc()
    return _cache["nc"]


def kernel(map1, map2):
    from concourse.bass_utils import run_bass_kernel_spmd

    nc = _get_nc()
    m1 = np.ascontiguousarray(np.asarray(map1, dtype=np.float32)).reshape(
        N_CORES, SPC, PART, FD
    )
    m2 = np.ascontiguousarray(np.asarray(map2, dtype=np.float32)).reshape(
        N_CORES, SPC, PART, FD
    )
    in_maps = [{"map1": m1[c], "map2": m2[c]} for c in range(N_CORES)]
    res = run_bass_kernel_spmd(nc, in_maps, list(range(N_CORES)))
    out = np.concatenate(
        [res.results[c]["out"].reshape(SPC, C, H, W) for c in range(N_CORES)],
        axis=0,
    )
    return out

